# revision 15
# baseline (speedup 1.0000x reference)
"""Single-head causal attention forward on 8 TRN2 NeuronCores.

Problem: x [8, 2048, 1024] f32, Wq/Wk/Wv [128, 1024] f32.
  q/k/v = x @ W.T ; S = q k^T / sqrt(128) causal ; out = softmax(S) v.

Sharding: data-parallel, one batch element per core (8 cores).

v2 design (output-transposed accumulation):
  - qT/kT/vT [h=128, t] via W-stationary matmuls; V natural [t, h] via PE
    transposes of vT.
  - S^T[j, q] tiles (kt-block stationary, qt moving, 512-col streams); exp
    on ScalarE with the 1/sqrt(128) scale folded in; diagonal 128x128
    blocks masked IN PLACE on DVE (tril kept via triu-mask multiply).
  - PV computes outT[h, q] = sum_j V_nat[j].T @ exp(S^T)[j] directly in one
    PSUM accumulation group per 512-token chunk: 40 long-stream matmuls
    instead of 136 short ones (fewer LDWEIGHTS, same MACs).
  - softmax denominators: DVE accumulates the exp tiles into an fp16
    accumulator; one ones-vector matmul per chunk reduces partitions.
    outT (unnormalized, bf16) and den (f32) are stored; the host divides
    and transposes during unsharding.
  - next chunk's projection matmuls are interleaved into the S/PV emission
    so the in-order PE queue never idles behind exp-paced PV ops.
  - DMA priority: wq + x chunk 0 issued first (sync+gpsimd queues), then
    wk/wv/tri, then x chunk 1; later chunks issued from inside the chunk
    bodies. ScalarE issues no DMA (it is exp-bound).
"""

import os
import sys

for _p in ("/opt/trn_rl_repo",):
    if _p not in sys.path and os.path.isdir(_p):
        sys.path.append(_p)

import numpy as np

B, T, D, H = 8, 2048, 1024, 128
CH = 512          # token chunk (free dim of S^T / outT tiles)
NCH = T // CH     # 4 chunks
CC = D // 128     # 8 contraction sub-tiles
NT = T // 128     # 16 token tiles
SCALE = 1.0 / np.sqrt(np.float32(H))

NWU = int(os.environ.get("KERNEL_NWU", "8"))  # warmup matmul count

_CACHE = {}


def _build():
    import concourse.bacc as bacc
    import concourse.mybir as mybir
    import concourse.tile as tile

    dt = mybir.dt
    bf16 = dt.bfloat16

    nc = bacc.Bacc(None)
    xh = nc.declare_dram_parameter("xh", [NCH, 128, CC, CH], bf16, isOutput=False)
    wqT = nc.declare_dram_parameter("wqT", [128, CC, H], bf16, isOutput=False)
    wkT = nc.declare_dram_parameter("wkT", [128, CC, H], bf16, isOutput=False)
    wvT = nc.declare_dram_parameter("wvT", [128, CC, H], bf16, isOutput=False)
    trieye = nc.declare_dram_parameter("trieye", [128, 256], bf16, isOutput=False)
    outT = nc.declare_dram_parameter("outT", [128, T], bf16, isOutput=True)
    den = nc.declare_dram_parameter("den", [1, T], dt.float32, isOutput=True)

    with tile.TileContext(nc) as tc:
        with (
            tc.tile_pool(name="singles", bufs=1) as singles,
            tc.tile_pool(name="xp", bufs=3) as xp,
            tc.tile_pool(name="qtp", bufs=2) as qtp,
            tc.tile_pool(name="ktp", bufs=4) as ktp,
            tc.tile_pool(name="vtp", bufs=2) as vtp,
            tc.tile_pool(name="ptp", bufs=6) as ptp,
            tc.tile_pool(name="accp", bufs=2) as accp,
            tc.tile_pool(name="outp", bufs=2) as outp,
            tc.tile_pool(name="denp", bufs=2) as denp,
            tc.tile_pool(name="psq", bufs=2, space="PSUM") as psq,
            tc.tile_pool(name="pss", bufs=4, space="PSUM") as pss,
            tc.tile_pool(name="pso", bufs=1, space="PSUM") as pso,
        ):
            # PE warmup: releases the HAM clock throttle (0.65->2.4GHz needs
            # ~3.4us of sustained PE activity) while the first DMAs land.
            wu_a = singles.tile([128, 128], bf16)
            wu_b = singles.tile([128, CH], bf16)
            nc.vector.memset(wu_a[:], 0.0)
            nc.vector.memset(wu_b[:], 0.0)
            wu_ps = pss.tile([128, CH], dt.float32, tag="sps")
            for i in range(NWU):
                nc.tensor.matmul(
                    wu_ps[:], wu_a[:], wu_b[:],
                    start=(i == 0), stop=(i == NWU - 1),
                )

            # --- DMA priority: wq + x0 first, then wk/wv/trieye, then x1 ---
            wq_sb = singles.tile([128, CC, H], bf16)
            wk_sb = singles.tile([128, CC, H], bf16)
            wv_sb = singles.tile([128, CC, H], bf16)
            te_sb = singles.tile([128, 256], bf16)
            tri_sb = te_sb[:, 0:128]
            eye_sb = te_sb[:, 128:256]

            def load_x(qc, pieces):
                # pieces: list of (engine, cc_start, cc_stop)
                xt = xp.tile([128, CC, CH], bf16, tag="xt")
                for eng, g0, g1 in pieces:
                    eng.dma_start(out=xt[:, g0:g1, :], in_=xh[qc, :, g0:g1, :])
                return xt

            # DMA rings process descriptors in order per engine queue, so
            # transfers queued BEHIND x0 on the same ring cannot steal its
            # bandwidth. wq + x0 head both rings; weights follow; x1 last.
            xts = [None] * NCH
            nc.sync.dma_start(out=wq_sb[:], in_=wqT[:])
            xts[0] = load_x(0, [
                (nc.gpsimd, 0, 2), (nc.gpsimd, 2, 4),
                (nc.sync, 4, 6), (nc.sync, 6, 8),
            ])
            nc.gpsimd.dma_start(out=wk_sb[:], in_=wkT[:])
            nc.gpsimd.dma_start(out=wv_sb[:], in_=wvT[:])
            nc.sync.dma_start(out=te_sb[:], in_=trieye[:])
            xts[1] = load_x(1, [(nc.sync, 0, 4), (nc.gpsimd, 4, 8)])

            # V natural [t, h] per j-tile; ones column vector for den reduce
            v_sb = singles.tile([128, NT, H], bf16)
            ones_sb = singles.tile([128, 128], bf16)
            nc.vector.memset(ones_sb[:], 1.0)

            w_sbs = {"q": wq_sb, "k": wk_sb, "v": wv_sb}

            def emit_proj(qc):
                """Generator of thunks: qkv projection + V transpose for
                chunk qc. Each yielded call emits one PE op (+ its attendant
                DVE/gpsimd ops)."""
                xt = xts[qc]
                outs = {}
                for which in ("q", "k", "v"):
                    ps = psq.tile([128, CH], dt.float32, tag="qk", name=f"{which}ps")
                    w = w_sbs[which]
                    for cc in range(CC):
                        yield lambda w=w, ps=ps, cc=cc: nc.tensor.matmul(
                            ps[:], w[:, cc, :], xt[:, cc, :],
                            start=(cc == 0), stop=(cc == CC - 1),
                        )
                    if which == "q":
                        t = qtp.tile([128, CH], bf16, name="qt")
                        eng = nc.vector
                    elif which == "k":
                        t = ktp.tile([128, CH], bf16, name="kt")
                        eng = nc.vector
                    else:
                        t = vtp.tile([128, CH], bf16, name="vt")
                        eng = nc.vector
                    outs[which] = t
                    yield lambda eng=eng, t=t, ps=ps: eng.tensor_copy(t[:], ps[:])
                vt = outs["v"]
                vtr = psq.tile([128, 4, 128], bf16, tag="qk", name="vtr")
                for ti in range(4):
                    yield lambda ti=ti, vtr=vtr: nc.tensor.transpose(
                        vtr[:, ti, :], vt[:, ti * 128 : (ti + 1) * 128], eye_sb
                    )
                yield lambda vtr=vtr: nc.vector.tensor_copy(
                    v_sb[:, qc * 4 : qc * 4 + 4, :], vtr[:]
                )
                outs_holder[qc] = outs

            outs_holder = {}
            kt_tiles = []

            # chunk 0's projection emitted directly (nothing to interleave
            # into); later chunks' projections interleave into the previous
            # chunk's S/PV phase.
            for th in emit_proj(0):
                th()
            kt_tiles.append(outs_holder[0]["k"])

            for qc in range(NCH):
                qt = outs_holder[qc]["q"]
                NJ = qc * 4 + 4

                # thunks of the NEXT chunk's projection, to interleave
                if qc + 1 < NCH:
                    nxt = emit_proj(qc + 1)
                else:
                    nxt = iter(())
                # per-j interleave budget: spread ~30 ops over NJ steps
                per_step = (34 // NJ) + 1

                # issue chunk qc+2's x load early in this chunk's phase
                if qc + 2 < NCH:
                    xts[qc + 2] = load_x(
                        qc + 2, [(nc.sync, 0, 4), (nc.gpsimd, 4, 8)]
                    )

                outp_ps = pso.tile([128, CH], dt.float32, tag="ovp")
                # den accumulator (DVE); the LAST diag tile is not added --
                # its contribution is folded into the den matmul directly
                acc = accp.tile([128, CH], bf16, name="acc")
                pts = [None] * NJ

                def emit_s(j):
                    diag = j >= qc * 4
                    v0 = (j - qc * 4) * 128 if diag else 0
                    sps = pss.tile([128, CH], dt.float32, tag="sps")
                    nc.tensor.matmul(
                        sps[:, v0:CH],
                        kt_tiles[j // 4][:, (j % 4) * 128 : (j % 4 + 1) * 128],
                        qt[:, v0:CH],
                        start=True, stop=True,
                    )
                    pt = ptp.tile([128, CH], bf16, tag="pt")
                    nc.scalar.activation(
                        pt[:, v0:CH], sps[:, v0:CH],
                        mybir.ActivationFunctionType.Exp, scale=float(SCALE),
                    )
                    if diag:
                        nc.vector.tensor_mul(
                            pt[:, v0 : v0 + 128], pt[:, v0 : v0 + 128], tri_sb
                        )
                    pts[j] = (pt, v0)

                PRE = 3
                for j in range(min(PRE, NJ)):
                    emit_s(j)

                for j in range(NJ):
                    pt, v0 = pts[j]
                    # den accumulation (masked diag values already in pt)
                    if j == 0:
                        nc.vector.tensor_copy(acc[:], pt[:])
                    elif j < NJ - 1:
                        nc.vector.tensor_add(
                            acc[:, v0:CH], acc[:, v0:CH], pt[:, v0:CH]
                        )
                    # PV accumulate into outT
                    nc.tensor.matmul(
                        outp_ps[:, v0:CH],
                        v_sb[:, j, :],
                        pt[:, v0:CH],
                        start=(j == 0), stop=(j == NJ - 1),
                        skip_group_check=True,
                    )
                    if j + PRE < NJ:
                        emit_s(j + PRE)
                    for _ in range(per_step):
                        th = next(nxt, None)
                        if th is not None:
                            th()

                for th in nxt:
                    th()
                if qc + 1 < NCH:
                    kt_tiles.append(outs_holder[qc + 1]["k"])

                # denominator: partition-reduce acc, plus the last diag
                # tile folded in directly (its DVE add is skipped so the
                # final den matmul only waits on exp+mask of the last tile)
                den_ps = pss.tile([128, CH], dt.float32, tag="dps", bufs=1)
                last_pt, last_v0 = pts[NJ - 1]
                nc.tensor.matmul(
                    den_ps[:], ones_sb[:], acc[:], start=True, stop=False
                )
                nc.tensor.matmul(
                    den_ps[:, last_v0:CH], ones_sb[:], last_pt[:, last_v0:CH],
                    start=False, stop=True,
                )
                den_sb = denp.tile([1, CH], dt.float32)
                if qc == NCH - 1:
                    nc.scalar.copy(den_sb[:], den_ps[0:1, :])
                else:
                    nc.vector.tensor_copy(den_sb[:], den_ps[0:1, :])
                den_eng = nc.scalar if qc == NCH - 1 else nc.gpsimd
                den_eng.dma_start(
                    out=den[0:1, qc * CH : (qc + 1) * CH], in_=den_sb[:]
                )

                # outT store (unnormalized, bf16)
                ot = outp.tile([128, CH], bf16)
                if qc == NCH - 1:
                    # tail: split the copy across two engines (scalar is idle
                    # by now), halving the last chunk's drain latency
                    nc.scalar.copy(ot[:, 0 : CH // 2], outp_ps[:, 0 : CH // 2])
                    nc.vector.tensor_copy(
                        ot[:, CH // 2 : CH], outp_ps[:, CH // 2 : CH]
                    )
                else:
                    nc.vector.tensor_copy(ot[:], outp_ps[:])
                nc.sync.dma_start(
                    out=outT[:, qc * CH : (qc + 1) * CH], in_=ot[:]
                )

    nc.compile()
    return nc


def _get_nc():
    if "nc" not in _CACHE:
        _CACHE["nc"] = _build()
    return _CACHE["nc"]


def _in_maps(x, Wq, Wk, Wv):
    import ml_dtypes

    bf16 = ml_dtypes.bfloat16

    def _wprep(W):
        # W [H, D] -> [128p, CC, H] with per-partition-contiguous rows
        WT = np.asarray(W, dtype=np.float32).T.reshape(CC, 128, H)
        return np.ascontiguousarray(WT.transpose(1, 0, 2)).astype(bf16)

    wq, wk, wv = _wprep(Wq), _wprep(Wk), _wprep(Wv)
    tri = np.triu(np.ones((128, 128), dtype=np.float32))
    eye = np.eye(128, dtype=np.float32)
    trieye = np.concatenate([tri, eye], axis=1).astype(bf16)
    x = np.asarray(x, dtype=np.float32)
    maps = []
    for b in range(B):
        # [qc, p, cc, t]: per (qc, p) a contiguous CC*CH run
        xhb = np.ascontiguousarray(
            x[b].T.reshape(CC, 128, NCH, CH).transpose(2, 1, 0, 3)
        ).astype(bf16)
        maps.append(
            {"xh": xhb, "wqT": wq, "wkT": wk, "wvT": wv, "trieye": trieye}
        )
    return maps


def kernel(x, Wq, Wk, Wv):
    from concourse.bass_utils import run_bass_kernel_spmd

    nc = _get_nc()
    res = run_bass_kernel_spmd(nc, _in_maps(x, Wq, Wk, Wv), core_ids=list(range(B)))
    outs = []
    for b in range(B):
        oT = np.asarray(res.results[b]["outT"], dtype=np.float32)  # [H, T]
        dn = np.asarray(res.results[b]["den"], dtype=np.float32)  # [1, T]
        outs.append((oT / dn).T)
    return np.stack(outs).astype(np.float32)
